# revision 21
# baseline (speedup 1.0000x reference)
"""Trainium2 Bass kernel for CapsuleLayer dynamic routing.

Problem: x [512, 1152, 8], W [1152, 10, 16, 8] -> v [512, 10, 16, 1]
  pred[b,p,n,t] = sum_d W[p,n,t,d] x[b,p,d]
  3 routing iterations; the b_ij update adds a batch-mean (keepdim) term, so
  b_ij is constant across batch => coupling coeffs are c[p,n] shared by all b.

Strategy (v3): the first collective on the 8-core stream cannot start
before ~67us (global-comm rendezvous + first-op setup, launch-anchored —
measured invariant to trigger time). That window is otherwise idle, so
iteration 0 — whose coupling coefficients are uniform (b=0) — is
REPLICATED: every core computes the FULL s0 = x @ W/N (contraction over
all 9216 pd, 288 fp16 matmuls, ~12.7MB input DMA) inside the window,
then squashes and runs its own P-shard's routing update locally. The
kernel therefore needs only TWO collectives on the critical path:
  iter1: s1-partial (own 144 prev-caps) -> AllReduce(fp16) -> squash
  iter2: s2-partial -> ReduceScatter(fp16) -> per-core output slice
The routing update is P-sharded throughout:
  M[pd,nt] = (1/B) sum_b x[b,pd] v[b,nt]    (PE)
  abar[p,n] = sum_{d,t} W2[pd,nt] M[pd,nt]  (DVE mul + reduce + S-matmul)
x2t/w2g carry all 72 pd-chunks with the core's OWN 9 chunks FIRST, so the
same [0:9) slice serves the sharded iterations on every core (SPMD-safe).

GEMM_DT "f16": 1 cycle/row PE matmuls (f32r is 4 cyc/row below free-dim
256), FWL weight loads, half the HBM/DVE/wire bytes, ~1e-3 rel err.
"""

import os
import sys

sys.path.insert(0, "/opt/trn_rl_repo")

import numpy as np

import concourse.bacc as bacc
import concourse.bass as bass
import concourse.mybir as mybir
import concourse.tile as tile
from concourse.bass_utils import run_bass_kernel_spmd

F32 = mybir.dt.float32
AF = mybir.ActivationFunctionType
ALU = mybir.AluOpType

B, P, N, T, D = 512, 1152, 10, 16, 8
NCORES = 8
PLOC = P // NCORES          # 144 prev caps per core
PD = PLOC * D               # 1152 local contraction length
CH = PD // 128              # 9 local chunks
CHF = (P * D) // 128        # 72 full-P chunks
BB = B // 128               # 4 batch blocks
NT = N * T                  # 160
NITER = 3

GEMM_DT = os.environ.get("CAPS_GEMM_DT", "f16")

_CACHE = {}


def _dt():
    return {"f32": F32, "f32r": mybir.dt.float32r,
            "bf16": mybir.dt.bfloat16, "f16": mybir.dt.float16}[GEMM_DT]


def _build():
    if "nc" in _CACHE:
        return _CACHE["nc"]

    DT = _dt()
    nc = bacc.Bacc("TRN2", target_bir_lowering=False, debug=False,
                   num_devices=NCORES)

    x2_d = nc.dram_tensor("x2", [128, BB, PD], DT, kind="ExternalInput")
    x2t_d = nc.dram_tensor("x2t", [128, CHF, B], DT, kind="ExternalInput")
    w2g_d = nc.dram_tensor("w2g", [128, CHF, NT], DT, kind="ExternalInput")
    smat_d = nc.dram_tensor("smat", [128, 16], F32, kind="ExternalInput")
    stmat_d = nc.dram_tensor("stmat", [16, 128], F32, kind="ExternalInput")
    vout_d = nc.dram_tensor("vout", [64, NT], F32, kind="ExternalOutput")

    rg = [list(range(NCORES))]
    NG = 9                  # x2t/w2g DMA pipeline groups
    GC = CHF // NG          # 8 chunks per group

    def squash(nc, wpool, s_full, lam, parts, blocks, tagp, out_dt,
               preload=None):
        """v = lam*s*f with f = sq/(1+sq*norm); s_full/[v] are
        [parts, blocks, NT]. Square/reduce/final-mul run in halves so they
        overlap the upstream copies and let the M-GEMM start early."""
        hb = blocks // 2 if blocks > 1 else blocks
        halves = [(0, hb), (hb, blocks)] if blocks > 1 else [(0, blocks)]
        # s^2 on ACT (Square lives in every table -> no reload); reduce on
        # DVE chases per half
        s2 = wpool.tile([parts, blocks, NT], F32, tag="s2" + tagp)
        sqr = wpool.tile([parts, blocks * N], F32, tag="sqr" + tagp)
        for lo, hi in halves:
            nc.scalar.activation(s2[:, lo:hi, :], s_full[:, lo:hi, :],
                                 AF.Square)
            nc.vector.tensor_reduce(
                sqr[:, lo * N:hi * N],
                s2[:, lo:hi, :].rearrange("p a (n t) -> p (a n) t", t=T),
                axis=mybir.AxisListType.X, op=ALU.add)
        norm = wpool.tile([parts, blocks * N], F32, tag="norm" + tagp)
        nc.scalar.activation(norm[:], sqr[:], AF.Sqrt, scale=lam * lam)
        if preload is not None:
            # anchor the Exp table prefetch on norm so the scheduler can't
            # hoist it to kernel start (where it would be useless)
            preload(AF.Exp, norm)
        # f*lam = lam^3*z / (1 + lam^3*z^1.5), z = sqr:
        #   fnum = lam^3 z (parallel w/ sqrt); nz = z*norm = lam*z^1.5;
        #   den = lam^2*nz + 1 (fused dual-op); fmul = fnum/den
        fnum = wpool.tile([parts, blocks * N], F32, tag="fnum" + tagp)
        nc.vector.tensor_scalar_mul(fnum[:], sqr[:], lam ** 3)
        nz = wpool.tile([parts, blocks * N], F32, tag="nz" + tagp)
        nc.vector.tensor_tensor(nz[:], sqr[:], norm[:], ALU.mult)
        den = wpool.tile([parts, blocks * N], F32, tag="den" + tagp)
        nc.vector.tensor_scalar(den[:], nz[:], lam * lam, 1.0,
                                ALU.mult, ALU.add)
        rden = wpool.tile([parts, blocks * N], F32, tag="rden" + tagp)
        nc.vector.reciprocal(rden[:], den[:])
        fmul = wpool.tile([parts, blocks * N], F32, tag="fmul" + tagp)
        nc.vector.tensor_tensor(fmul[:], fnum[:], rden[:], ALU.mult)
        v = wpool.tile([parts, blocks, NT], out_dt, tag="v" + tagp)
        for lo, hi in halves:
            nc.vector.tensor_tensor(
                v[:, lo:hi, :].rearrange("p a (n t) -> p a n t", t=T),
                s_full[:, lo:hi, :].rearrange("p a (n t) -> p a n t", t=T),
                fmul[:, lo * N:hi * N].rearrange("p (a n) -> p a n", n=N)
                    .unsqueeze(3).broadcast_to([parts, hi - lo, N, T]),
                ALU.mult)
        return v

    with tile.TileContext(nc) as tc:
        with (
            tc.tile_pool(name="const", bufs=1) as cpool,
            tc.tile_pool(name="work", bufs=2) as wpool,
            tc.tile_pool(name="ps_s", bufs=4, space="PSUM") as ps_s,
            tc.tile_pool(name="ps_m", bufs=2, space="PSUM") as ps_m,
            tc.tile_pool(name="dram", bufs=2, space="DRAM") as dpool,
        ):
            # early dummy collective, entirely on the gpsimd queue: it
            # triggers ~10us in and absorbs the one-time global-comm
            # rendezvous + first-op setup (~30us) while the real phase-0
            # DMA/GEMM runs; the real AllReduce then starts trigger+1.2us
            # instead of trigger+11.5us.
            warm_src = cpool.tile([1, 64], F32, tag="warm_src")
            nc.gpsimd.memset(warm_src[:], 0.0)
            cc_w_in = dpool.tile([1, 64], F32, tag="ccw_in")
            nc.gpsimd.dma_start(cc_w_in[:], warm_src[:])
            cc_w_out = dpool.tile([NCORES, 64], F32, tag="ccw_out")
            nc.gpsimd.collective_compute(
                "AllGather", ALU.bypass, replica_groups=rg,
                ins=[cc_w_in.opt()], outs=[cc_w_out.opt()])

            smat = cpool.tile([128, 16], F32)
            nc.sync.dma_start(smat[:], smat_d[:])
            stmat = cpool.tile([16, 128], F32)
            nc.sync.dma_start(stmat[:], stmat_d[:])
            # grouped loads so the iter-0 matmuls chase the input DMA: a
            # small first group gets the PE started early; x2t rides the
            # sync HWDGE queue, w2g the scalar one (parallel issue); x2
            # (needed by the update ~60us in) loads after everything else
            x2t = cpool.tile([128, CHF, B], DT, tag="x2t")
            w2g = cpool.tile([128, CHF, NT], DT, tag="w2g")
            x2 = cpool.tile([128, BB, PD], DT)
            bounds = [0, 2, 6, 14, 22, 30, 38, 46, 54, 63, 72]
            for g in range(len(bounds) - 1):
                sl = slice(bounds[g], bounds[g + 1])
                nc.sync.dma_start(x2t[:, sl, :], x2t_d[:, sl, :])
                nc.scalar.dma_start(w2g[:, sl, :], w2g_d[:, sl, :])
            for bb in range(BB):
                nc.sync.dma_start(x2[:, bb, :], x2_d[:, bb, :])

            act_scr = cpool.tile([1, 4], F32, tag="act_scr")

            def act_preload(func, anchor=None):
                # dummy activation: pulls the ACT function-table reload off
                # the next real activation's critical path. `anchor` gives
                # it a data dependency so the scheduler places it right.
                src = smat if anchor is None else anchor
                nc.scalar.activation(act_scr[:], src[0:1, 0:4], func)

            act_preload(AF.Sqrt)

            lam = 1.0 / N       # iteration 0: uniform c folded via lam
            wct = None          # [0:CH) Wc chunks for sharded iterations
            bbar = None

            for it in range(NITER):
                # ---- s accumulation. it==0: FULL contraction (replicated
                # on every core, inside the collective-rendezvous window);
                # it>=1: own 144 prev-caps only.
                nch = CHF if it == 0 else CH
                s_sb = wpool.tile([128, BB, NT], DT, tag="s_sb")
                s_ps = [ps_s.tile([128, NT], F32, name=f"s_ps{bb}",
                                  tag=f"s_ps{bb}", bufs=1)
                        for bb in range(BB)]
                for c in range(nch):
                    wcc = w2g[:, c, :] if it == 0 else wct[c][:]
                    for bb in range(BB):
                        nc.tensor.matmul(
                            s_ps[bb][:],
                            x2t[:, c, bb * 128:(bb + 1) * 128],
                            wcc, start=(c == 0), stop=(c == nch - 1))

                if it == 0:
                    # no collective: s0 is complete locally. psum->sbuf
                    # copies per half (ACT + DVE) so the squash chases.
                    s_full = wpool.tile([128, BB, NT], DT, tag="s_full")
                    for half in range(2):
                        b0 = 2 * half
                        nc.scalar.activation(s_full[:, b0, :],
                                             s_ps[b0][:], AF.Copy)
                        nc.vector.tensor_copy(s_full[:, b0 + 1, :],
                                              s_ps[b0 + 1][:])
                else:
                    # psum->sbuf copies split across ACT and DVE; bounce
                    # DMA per half fires as soon as its copies land
                    cc_in = dpool.tile([128, BB, NT], DT, tag="cc_in")
                    for half in range(2):
                        b0 = 2 * half
                        nc.scalar.activation(s_sb[:, b0, :], s_ps[b0][:],
                                             AF.Copy)
                        nc.vector.tensor_copy(s_sb[:, b0 + 1, :],
                                              s_ps[b0 + 1][:])
                        nc.sync.dma_start(cc_in[:, b0:b0 + 2, :],
                                          s_sb[:, b0:b0 + 2, :])

                    if it == NITER - 1:
                        # ---- final: ReduceScatter; each core outputs its
                        # partition slice, the host assembles
                        cc_rs = dpool.tile([16, BB, NT], DT, tag="cc_rs")
                        nc.gpsimd.collective_compute(
                            "ReduceScatter", ALU.add, replica_groups=rg,
                            ins=[cc_in.opt()], outs=[cc_rs.opt()])
                        s_last = wpool.tile([64, 1, NT], DT, tag="s_last")
                        nc.sync.dma_start(
                            s_last.rearrange("p a f -> (p a) f"),
                            cc_rs.rearrange("p a f -> (p a) f"))
                        v = squash(nc, wpool, s_last, lam, 64, 1, "L", F32)
                        nc.sync.dma_start(vout_d[:],
                                          v.rearrange("p a f -> (p a) f"))
                        break

                    # ---- AllReduce partial s over the 8 P-shards
                    cc_out = dpool.tile([128, BB, NT], DT, tag="cc_out")
                    nc.gpsimd.collective_compute(
                        "AllReduce", ALU.add, replica_groups=rg,
                        ins=[cc_in.opt()], outs=[cc_out.opt()])
                    s_full = wpool.tile([128, BB, NT], DT, tag="s_full")
                    nc.sync.dma_start(s_full[:, 0:2, :], cc_out[:, 0:2, :])
                    nc.sync.dma_start(s_full[:, 2:4, :], cc_out[:, 2:4, :])

                # throwaway matmuls gated on s_full: they run during the
                # squash chain so the PE doesn't drop to its HAM-throttled
                # half-clock state before the M-GEMM burst
                warm_ps = ps_s.tile([128, NT], F32, name=f"warm{it}",
                                    tag="s_ps0", bufs=1)
                for _ in range(6 if it == 0 else 10):
                    nc.tensor.matmul(warm_ps[:], s_full[:, 0, 0:128],
                                     s_full[:, 0, :], start=True, stop=True)

                v_g = squash(nc, wpool, s_full, lam, 128, BB, f"i{it}", DT,
                             preload=act_preload)

                # ---- routing update (always on the core's own P-shard)
                # M[pd, nt] = sum_b x2[b, pd] v[b, nt]   (1/B folded in smat)
                rtile = wpool.tile([128, CH * N], F32, tag="rtile")
                for c in range(CH):
                    m_ps = ps_m.tile([128, NT], F32, tag="m_ps")
                    for bb in range(BB):
                        nc.tensor.matmul(
                            m_ps[:], x2[:, bb, c * 128:(c + 1) * 128],
                            v_g[:, bb, :], start=(bb == 0),
                            stop=(bb == BB - 1))
                    e_sb = wpool.tile([128, NT], DT, tag="e_sb", bufs=3)
                    if c % 2 == 0:
                        # even chunks: DVE multiplies straight from PSUM
                        nc.vector.tensor_tensor(
                            e_sb[:], w2g[:, c, :], m_ps[:], ALU.mult)
                    else:
                        # odd chunks: ACT (tableless Copy) unloads PSUM and
                        # GpSimd multiplies, freeing DVE for the reduces
                        m_sb = wpool.tile([128, NT], DT, tag="m_sb", bufs=2)
                        nc.scalar.activation(m_sb[:], m_ps[:], AF.Copy)
                        nc.gpsimd.tensor_tensor(
                            e_sb[:], w2g[:, c, :], m_sb[:], ALU.mult)
                    nc.vector.tensor_reduce(
                        rtile[:, c * N:(c + 1) * N],
                        e_sb.rearrange("p (n t) -> p n t", t=T),
                        axis=mybir.AxisListType.X, op=ALU.add)

                # abar[pl, (c,n)] = sum_d R[(pl,d), (c,n)] / B   via smat
                a_ps = ps_m.tile([16, CH * N], F32, tag="a_ps", bufs=1)
                nc.tensor.matmul(a_ps[:], smat[:], rtile[:],
                                 start=True, stop=True)

                bnew = wpool.tile([16, CH * N], F32, tag="bbar")
                eb = wpool.tile([16, CH * N], F32, tag="eb")
                if bbar is None:
                    # keep the copy off the critical path: exp straight from
                    # PSUM, the persistent b copy happens in parallel
                    nc.scalar.activation(eb[:], a_ps[:], AF.Exp)
                    nc.vector.tensor_copy(bnew[:], a_ps[:])
                else:
                    nc.vector.tensor_tensor(bnew[:], bbar[:], a_ps[:],
                                            ALU.add)
                    nc.scalar.activation(eb[:], bnew[:], AF.Exp)
                bbar = bnew
                act_preload(AF.Sqrt, eb)
                ssum = wpool.tile([16, CH], F32, tag="ssum")
                nc.vector.tensor_reduce(
                    ssum[:], eb.rearrange("p (c n) -> p c n", n=N),
                    axis=mybir.AxisListType.X, op=ALU.add)
                rsum = wpool.tile([16, CH], F32, tag="rsum")
                nc.vector.reciprocal(rsum[:], ssum[:])
                cb16 = wpool.tile([16, CH * N], F32, tag="cb16")
                nc.vector.tensor_tensor(
                    cb16.rearrange("p (c n) -> p c n", n=N),
                    eb.rearrange("p (c n) -> p c n", n=N),
                    rsum.unsqueeze(2).broadcast_to([16, CH, N]),
                    ALU.mult)

                # broadcast c over d: cb[(pl,d), (c,n)] via stmat
                cb_ps = ps_m.tile([128, CH * N], F32, tag="cb_ps", bufs=1)
                nc.tensor.matmul(cb_ps[:], stmat[:], cb16[:],
                                 start=True, stop=True)
                cb = wpool.tile([128, CH * N], DT, tag="cb")
                nc.vector.tensor_copy(cb[:], cb_ps[:])

                # Wc_c = W2_c * c (broadcast over t), own chunks only
                wc_new = []
                for c in range(CH):
                    wcn = wpool.tile([128, NT], DT, tag=f"wct{c}")
                    nc.vector.tensor_tensor(
                        wcn.rearrange("p (n t) -> p n t", t=T),
                        w2g[:, c, :].rearrange("p (n t) -> p n t", t=T),
                        cb[:, c * N:(c + 1) * N]
                            .unsqueeze(2).broadcast_to([128, N, T]),
                        ALU.mult)
                    wc_new.append(wcn)
                wct = wc_new
                lam = 1.0

    nc.compile()
    _CACHE["nc"] = nc
    return nc


def _round_f32r(a):
    # round-to-nearest-even keeping 11 mantissa bits (top 20 bits of fp32)
    u = np.ascontiguousarray(a, dtype=np.float32).view(np.uint32)
    keep = np.uint32(0xFFFFF000)
    bit = (u >> np.uint32(12)) & np.uint32(1)
    return ((u + np.uint32(0x7FF) + bit) & keep).view(np.float32)


def _cast(a):
    if GEMM_DT == "f32":
        return np.ascontiguousarray(a, dtype=np.float32)
    if GEMM_DT == "f32r":
        return _round_f32r(np.ascontiguousarray(a, dtype=np.float32))
    if GEMM_DT == "f16":
        return np.ascontiguousarray(a, dtype=np.float16)
    import ml_dtypes
    return np.ascontiguousarray(a).astype(ml_dtypes.bfloat16)


def _prep_inputs(x, W):
    x = np.ascontiguousarray(x, dtype=np.float32)
    W = np.ascontiguousarray(W, dtype=np.float32)
    # smat[pl*8+d, pl] = 1/B ; stmat[pl, pl*8+d] = 1
    smat = np.kron(np.eye(16, dtype=np.float32),
                   np.ones((D, 1), np.float32)) / float(B)   # [128, 16]
    stmat = np.kron(np.eye(16, dtype=np.float32),
                    np.ones((1, D), np.float32))             # [16, 128]
    # full-P chunk layouts (chunk g covers flat pd [g*128, (g+1)*128))
    xt_all = x.reshape(B, P * D).T.reshape(CHF, 128, B)       # [72,128,b]
    w_all = W.transpose(0, 3, 1, 2).reshape(CHF, 128, NT)     # [72,128,nt]
    in_maps = []
    for k in range(NCORES):
        # own 9 chunks first so the sharded iterations can use [0:CH) on
        # every core (SPMD: no core-dependent indexing inside the kernel)
        own = list(range(k * CH, (k + 1) * CH))
        rest = [g for g in range(CHF) if g not in own]
        order = own + rest
        x2t = np.ascontiguousarray(
            xt_all[order].transpose(1, 0, 2))                 # [128,72,b]
        w2g = np.ascontiguousarray(
            w_all[order].transpose(1, 0, 2))                  # [128,72,nt]
        xk = x[:, k * PLOC:(k + 1) * PLOC, :].reshape(B, PD)  # [b, pdl]
        x2 = np.ascontiguousarray(
            xk.reshape(BB, 128, PD).transpose(1, 0, 2))       # [128,4,pdl]
        in_maps.append({
            "x2": _cast(x2), "x2t": _cast(x2t), "w2g": _cast(w2g),
            "smat": smat, "stmat": stmat,
        })
    return in_maps


def run(x, W, trace=False):
    nc = _build()
    in_maps = _prep_inputs(x, W)
    res = run_bass_kernel_spmd(nc, in_maps, list(range(NCORES)), trace=trace)
    # each core k returns the summed partition slice [16k:16k+16] of
    # [128, BB, NT]; assemble, then b = bb*128 + p
    vfull = np.empty((128, BB, NT), dtype=np.float32)
    for k in range(NCORES):
        vfull[16 * k:16 * (k + 1)] = res.results[k]["vout"].reshape(16, BB, NT)
    v = vfull.transpose(1, 0, 2).reshape(B, N, T)
    out = np.ascontiguousarray(v[..., None], dtype=np.float32)
    return out, res.exec_time_ns


def kernel(x, W):
    return run(x, W, trace=False)[0]
